# revision 8
# baseline (speedup 1.0000x reference)
"""Graphormer layer on 8 TRN2 NeuronCores.

Sharding: core c handles batch b = c//2 and query-row half qh = c%2 (1024 q
rows). All compute is in transposed (feature-on-partition) layout; the host
pre-transposes x and the influence slices and transposes per-core outputs
back during the gather. Host also rolls the node axis per core so each core's
own q rows sit at columns [0, 1024) — the device program is identical across
cores (attention over all keys is permutation-invariant; the influence k axis
is rolled identically).

Stage-E engine balance: per k-chunk a path is chosen from
  'A': influence bias preloaded into PSUM by PE identity-matmul, exact exp
       on the ACT engine.
  'S': same PSUM preload, exp approximated on the Vector engine with a
       Schraudolph bf16 bit-trick (tensor_scalar -> int16, bitcast bf16).
  'P': plain scores; exp on ACT, then multiply by EG = exp(LG) on DVE
       (no PE preload).
The per-(k,q) gate G2 multiply (f = e * G2) always runs on DVE (bf16 2x),
with a configurable subset routed to GpSimd.
"""

import math

import numpy as np
import ml_dtypes

import concourse.bass as bass
import concourse.bacc as bacc
import concourse.mybir as mybir
import concourse.tile as tile
from concourse.bass_utils import run_bass_kernel_spmd

B, N, E, H, D = 4, 2048, 256, 8, 32
NQ = N // 2          # q rows per core
QC = 512             # q window
NKC = N // 128       # 16 k-chunks
EC = E // 128        # 2 feature chunks

f32 = mybir.dt.float32
bf16 = mybir.dt.bfloat16
i16 = mybir.dt.int16
FT = mybir.ActivationFunctionType
ALU = mybir.AluOpType

# Schraudolph constants for bf16 bitcast exp: e ~= bitcast(i16(x*SA + SB))
SA = 128.0 / math.log(2.0)
C_ADJ = 6.0
SB = 127.0 * 128.0 - C_ADJ

# per-kc exp path: 'A' (ACT exp, LG in psum), 'S' (DVE schraudolph, LG in
# psum), 'P' (ACT exp, EG multiply on DVE)
PATHS = "ASAPAASAPAASAPAA"
assert len(PATHS) == NKC
# route every FG-th f-multiply to gpsimd (0 = never)
F_GPS_EVERY = 8

# vecs_sb column index: vec v, chunk c -> 2*v + c
V_G1, V_BETA1, V_G2, V_BETA2, V_BO, V_B1, V_B2 = range(7)
# scal columns: iw1, ib1, iw2, ib2
S_IW1, S_IB1, S_IW2, S_IB2 = range(4)


def build_body(nc, tc, xT_d, xTb_d, inflT_d, w_d, vecs_d, scal_d, ident_d,
               outT_d):
    persist_pools = []

    def ppool(name):
        p = tc.tile_pool(name=name, bufs=1)
        persist_pools.append(p)
        return p.__enter__()

    persist = ppool("persist")

    # ---- persistent SBUF ----
    qt = [persist.tile([128, NQ], bf16, name=f"qt{c}", tag=f"qt{c}") for c in range(EC)]
    kt = [persist.tile([128, N], bf16, name=f"kt{c}", tag=f"kt{c}") for c in range(EC)]
    xt = [persist.tile([128, N], f32, name=f"xt{c}", tag=f"xt{c}") for c in range(EC)]
    xb = [persist.tile([128, N], bf16, name=f"xb{c}", tag=f"xb{c}") for c in range(EC)]
    ln1 = [persist.tile([128, N], bf16, name=f"ln1{c}", tag=f"ln1{c}") for c in range(EC)]
    v_sb = [persist.tile([128, E], bf16, name=f"v{k}", tag=f"v{k}") for k in range(NKC)]
    ga_sb = [persist.tile([128, NQ], bf16, name=f"ga_{k}", tag=f"ga_{k}") for k in range(NKC)]
    gb_sb = [persist.tile([128, NQ], bf16, name=f"gb_{k}", tag=f"gb_{k}") for k in range(NKC)]
    id_bf = persist.tile([128, 128], bf16, name="id_bf", tag="id_bf")
    w_sb = {n: persist.tile([128, 2 * E], bf16, name=f"w_{n}", tag=f"w_{n}") for n in w_d}
    vecs = persist.tile([128, 14], f32, name="vecs", tag="vecs")
    scal = persist.tile([128, 4], f32, name="scal", tag="scal")
    ones = persist.tile([128, 128], f32, name="ones", tag="ones")
    ones_bf = persist.tile([128, 32], bf16, name="ones_bf", tag="ones_bf")
    h_sb = [[persist.tile([128, QC], f32, name=f"h{q}{c}", tag=f"h{q}{c}") for c in range(EC)]
            for q in range(2)]
    eps_t = persist.tile([128, 1], f32, name="eps_t", tag="eps_t")
    warm = persist.tile([128, 8], f32, name="warm", tag="warm")

    # ---- ACT table warmup: force exp table load before any data arrives ----
    nc.vector.memset(warm[:, 0:4], 0.0)
    nc.scalar.activation(warm[:, 4:8], warm[:, 0:4], FT.Exp)

    # ---- small loads ----
    for n in w_d:
        for c in range(EC):
            nc.sync.dma_start(w_sb[n][:, E * c:E * (c + 1)],
                              w_d[n][128 * c:128 * (c + 1), :])
    nc.sync.dma_start(vecs[:, :], vecs_d[:, :])
    nc.sync.dma_start(scal[:, :], scal_d[:, :])
    idt = persist.tile([128, 128], f32, name="id_f32", tag="id_f32")
    nc.sync.dma_start(idt[:, :], ident_d[:, :])
    nc.vector.tensor_copy(id_bf[:, :], idt[:, :])
    nc.vector.memset(eps_t[:, :], 1e-5)
    nc.vector.memset(ones[:, :], 1.0)
    nc.vector.memset(ones_bf[:, :], 1.0)

    # ---- x loads (per 512-window so LN can start early) ----
    for w in range(N // 512):
        for c in range(EC):
            nc.sync.dma_start(xt[c][:, 512 * w:512 * (w + 1)],
                              xT_d[128 * c:128 * (c + 1), 512 * w:512 * (w + 1)])
            nc.sync.dma_start(xb[c][:, 512 * w:512 * (w + 1)],
                              xTb_d[128 * c:128 * (c + 1), 512 * w:512 * (w + 1)])

    # ---- influence loads (bf16, per k-chunk) ----
    u_pool = tc.tile_pool(name="u_pool", bufs=3)
    persist_pools.append(u_pool)
    up = u_pool.__enter__()
    u_tiles = {}
    for k in range(NKC):
        u = up.tile([128, NQ], bf16, name=f"u{k}", tag="u")
        nc.sync.dma_start(u[:, :], inflT_d[128 * k:128 * (k + 1), :])
        u_tiles[k] = u

    # ---- shared PSUM pools (4 banks score/general + 4 banks accumulators) --
    ps_pool = tc.tile_pool(name="ps", bufs=2, space="PSUM")
    persist_pools.append(ps_pool)
    ps = ps_pool.__enter__()
    acc_pool = tc.tile_pool(name="acc", bufs=1, space="PSUM")
    persist_pools.append(acc_pool)
    accp = acc_pool.__enter__()
    sbuf_pool = tc.tile_pool(name="work", bufs=1)
    persist_pools.append(sbuf_pool)
    wk = sbuf_pool.__enter__()
    ef_pool = tc.tile_pool(name="ef", bufs=3)
    persist_pools.append(ef_pool)
    efp = ef_pool.__enter__()
    io_pool = tc.tile_pool(name="io", bufs=2)
    persist_pools.append(io_pool)
    iop = io_pool.__enter__()

    ones_lnb = persist.tile([128, 128], bf16, name="ones_lnb", tag="ones_lnb")
    nc.vector.memset(ones_lnb[:, :], 1.0)

    def layer_norm_T(x_chunks, win, wn, g_col, b_col, out_chunks, in_f32):
        """T-layout LN over partition dim; rstd via exp(-0.5*ln(var+eps))."""
        sdt = f32 if in_f32 else bf16
        ones_mm = ones if in_f32 else ones_lnb
        p_s = ps.tile([128, wn], f32, name="lnps", tag="ps")
        for c in range(EC):
            nc.tensor.matmul(p_s[:, :wn], ones_mm[:, :],
                             x_chunks[c][:, win:win + wn],
                             start=(c == 0), stop=(c == EC - 1))
        mu = wk.tile([128, wn], sdt, name="lnmu", tag="lnmu")
        nc.vector.tensor_scalar_mul(mu[:, :], p_s[:, :wn], 1.0 / E)
        mu2 = wk.tile([128, wn], f32, name="lnmu2", tag="lnmu2")
        nc.scalar.activation(mu2[:, :], mu[:, :], FT.Square)
        sq = wk.tile([128, 2 * wn], sdt, name="lnsq", tag="lnsq")
        p_sq = ps.tile([128, wn], f32, name="lnpsq", tag="ps")
        for c in range(EC):
            nc.scalar.activation(sq[:, c * wn:(c + 1) * wn],
                                 x_chunks[c][:, win:win + wn], FT.Square)
            nc.tensor.matmul(p_sq[:, :wn], ones_mm[:, :],
                             sq[:, c * wn:(c + 1) * wn],
                             start=(c == 0), stop=(c == EC - 1))
        msq = wk.tile([128, wn], f32, name="lnmsq", tag="lnmsq")
        nc.vector.tensor_scalar_mul(msq[:, :], p_sq[:, :wn], 1.0 / E)
        var = wk.tile([128, wn], f32, name="lnvar", tag="lnvar")
        nc.vector.tensor_sub(var[:, :], msq[:, :], mu2[:, :])
        lnv = wk.tile([128, wn], f32, name="lnlnv", tag="lnlnv")
        nc.scalar.activation(lnv[:, :], var[:, :], FT.Ln, bias=eps_t[:, :])
        rstd = wk.tile([128, wn], sdt, name="lnrstd", tag="lnrstd")
        nc.scalar.activation(rstd[:, :], lnv[:, :], FT.Exp, scale=-0.5)
        for c in range(EC):
            xs = x_chunks[c][:, win:win + wn]
            xm = wk.tile([128, wn], sdt, name="lnxm", tag="lnxm")
            nc.vector.tensor_sub(xm[:, :], xs, mu[:, :])
            xm2 = wk.tile([128, wn], sdt, name="lnxm2", tag="lnxm2")
            nc.vector.tensor_mul(xm2[:, :], xm[:, :], rstd[:, :])
            nc.vector.tensor_scalar(
                out_chunks[c][:, win:win + wn], xm2[:, :],
                vecs[:, 2 * g_col + c:2 * g_col + c + 1],
                vecs[:, 2 * b_col + c:2 * b_col + c + 1],
                ALU.mult, ALU.add)

    # ---- prologue: LN1 w0,w1 -> Q proj; then per kw: K, V, preps ----
    for w in range(2):
        layer_norm_T(xb, 512 * w, 512, V_G1, V_BETA1, ln1, in_f32=False)
    for fc in range(EC):
        for qw in range(NQ // 512):
            pq = ps.tile([128, 512], f32, name="proj", tag="ps")
            for ec in range(EC):
                nc.tensor.matmul(
                    pq[:, :],
                    w_sb["Wq"][:, E * ec + 128 * fc:E * ec + 128 * (fc + 1)],
                    ln1[ec][:, 512 * qw:512 * (qw + 1)],
                    start=(ec == 0), stop=(ec == EC - 1))
            nc.vector.tensor_copy(qt[fc][:, 512 * qw:512 * (qw + 1)], pq[:, :])

    def prep_kc(k):
        u = u_tiles[k]
        path = PATHS[k]
        if path in "AS":
            nc.vector.tensor_scalar(ga_sb[k][:, :], u[:, :], scal[:, 0:1],
                                    scal[:, 1:2], ALU.mult, ALU.add)
        else:  # 'P': EG = exp(iw1*u + ib1)
            nc.scalar.activation(ga_sb[k][:, :], u[:, :], FT.Exp,
                                 scale=scal[:, 0:1], bias=scal[:, 1:2])
        nc.vector.tensor_scalar(gb_sb[k][:, :], u[:, :], scal[:, 2:3],
                                scal[:, 3:4], ALU.mult, ALU.add)

    for kw in range(4):
        if kw >= 2:
            layer_norm_T(xb, 512 * kw, 512, V_G1, V_BETA1, ln1, in_f32=False)
        for fc in range(EC):
            pk = ps.tile([128, 512], f32, name="proj", tag="ps")
            for ec in range(EC):
                nc.tensor.matmul(
                    pk[:, :],
                    w_sb["Wk"][:, E * ec + 128 * fc:E * ec + 128 * (fc + 1)],
                    ln1[ec][:, 512 * kw:512 * (kw + 1)],
                    start=(ec == 0), stop=(ec == EC - 1))
            nc.vector.tensor_copy(kt[fc][:, 512 * kw:512 * (kw + 1)], pk[:, :])
        for k in range(4 * kw, 4 * kw + 4):
            pv = ps.tile([128, E], f32, name="projv", tag="ps")
            for ec in range(EC):
                nc.tensor.matmul(
                    pv[:, :],
                    ln1[ec][:, 128 * k:128 * (k + 1)],
                    w_sb["Wv"][:, E * ec:E * (ec + 1)],
                    start=(ec == 0), stop=(ec == EC - 1))
            nc.vector.tensor_copy(v_sb[k][:, :], pv[:, :])
            prep_kc(k)

    # ---- attention + per-qc epilogue/FFN ----
    tile_idx = 0
    for qc in range(2):
        q0 = QC * qc
        wv_ps = [accp.tile([128, QC], f32, name=f"wv{qc}{s}", tag=f"wv{s}")
                 for s in range(2)]
        z_ps = [accp.tile([128, QC], f32, name=f"z{qc}{s}", tag=f"z{s}")
                for s in range(2)]
        for kc in range(NKC):
            path = PATHS[kc]
            gab = ga_sb[kc][:, q0:q0 + QC].rearrange(
                "p (o q) -> p o q", o=1).broadcast_to([128, 2, QC])
            gbb = gb_sb[kc][:, q0:q0 + QC].rearrange(
                "p (o q) -> p o q", o=1).broadcast_to([128, 2, QC])
            for half in range(2):
                for hg in (2 * half, 2 * half + 1):
                    st = ps.tile([128, 2 * QC], f32, name="score", tag="ps")
                    if path in "AS":
                        for j in range(2):
                            nc.tensor.matmul(
                                st[:, QC * j:QC * (j + 1)],
                                id_bf[:, :],
                                ga_sb[kc][:, q0:q0 + QC],
                                start=True, stop=False)
                    for j in range(2):
                        h = 2 * hg + j
                        c, hh = h // 4, 32 * (h % 4)
                        nc.tensor.matmul(
                            st[:, QC * j:QC * (j + 1)],
                            kt[c][hh:hh + 32, 128 * kc:128 * (kc + 1)],
                            qt[c][hh:hh + 32, q0:q0 + QC],
                            start=(path == "P"), stop=True,
                            skip_group_check=True, tile_position=(hh, 0))
                    # exp / gating per path
                    if path == "A":
                        e = efp.tile([128, 2 * QC], bf16, name="e", tag="e")
                        nc.scalar.activation(e[:, :], st[:, :], FT.Exp)
                        ztile, zcast = e, False
                    elif path == "S":
                        ei = efp.tile([128, 2 * QC], i16, name="es", tag="e")
                        nc.vector.tensor_scalar(ei[:, :], st[:, :], SA, SB,
                                                ALU.mult, ALU.add)
                        ztile, zcast = ei, True
                    else:  # 'P'
                        e0 = efp.tile([128, 2 * QC], bf16, name="e", tag="e")
                        nc.scalar.activation(e0[:, :], st[:, :], FT.Exp)
                        zt = efp.tile([128, 2 * QC], bf16, name="zt", tag="zt")
                        nc.vector.tensor_tensor(
                            zt[:, :].rearrange("p (o q) -> p o q", o=2),
                            e0[:, :].rearrange("p (o q) -> p o q", o=2),
                            gab, ALU.mult)
                        ztile, zcast = zt, False

                    def zsl(j):
                        ap = ztile[:, QC * j:QC * (j + 1)]
                        return ap.bitcast(bf16) if zcast else ap

                    f = efp.tile([128, 2 * QC], bf16, name="f", tag="f")
                    fsrc = ztile[:, :].bitcast(bf16) if zcast else ztile[:, :]
                    feng = nc.vector
                    tile_idx += 1
                    if F_GPS_EVERY and tile_idx % F_GPS_EVERY == 0:
                        feng = nc.gpsimd
                    feng.tensor_tensor(
                        f[:, :].rearrange("p (o q) -> p o q", o=2),
                        fsrc.rearrange("p (o q) -> p o q", o=2),
                        gbb, ALU.mult)
                    for j in range(2):
                        h = 2 * hg + j
                        s_, hh = h // 4, 32 * (h % 4)
                        nc.tensor.matmul(
                            z_ps[s_][hh:hh + 32, :],
                            ones_bf[:, :],
                            zsl(j),
                            start=(kc == 0), stop=(kc == NKC - 1),
                            skip_group_check=True, tile_position=(0, hh))
                        nc.tensor.matmul(
                            wv_ps[s_][hh:hh + 32, :],
                            v_sb[kc][:, 32 * h:32 * h + 32],
                            f[:, QC * j:QC * (j + 1)],
                            start=(kc == 0), stop=(kc == NKC - 1),
                            skip_group_check=True, tile_position=(0, hh))
        # ---- epilogue: normalize + Wo + residual -> h ----
        on = []
        for s in range(2):
            zr = wk.tile([128, QC], f32, name=f"zr{s}", tag=f"zr{s}")
            nc.vector.reciprocal_approx_fast(zr[:, :], z_ps[s][:, :])
            o = wk.tile([128, QC], bf16, name=f"on{s}", tag=f"on{s}")
            nc.vector.tensor_mul(o[:, :], wv_ps[s][:, :], zr[:, :])
            on.append(o)
        for fc in range(EC):
            po = ps.tile([128, QC], f32, name="po", tag="ps")
            for ec in range(EC):
                nc.tensor.matmul(
                    po[:, :],
                    w_sb["Wo"][:, E * ec + 128 * fc:E * ec + 128 * (fc + 1)],
                    on[ec][:, :],
                    start=(ec == 0), stop=(ec == EC - 1))
            nc.vector.affine_then_add(
                h_sb[qc][fc][:, :], po[:, :], xt[fc][:, q0:q0 + QC],
                1.0, vecs[:, 2 * V_BO + fc:2 * V_BO + fc + 1])
        # ---- stage F: LN2 + FFN + residual + store ----
        ln2 = [wk.tile([128, QC], bf16, name=f"ln2{c}", tag=f"ln2{c}")
               for c in range(EC)]
        layer_norm_T(h_sb[qc], 0, QC, V_G2, V_BETA2, ln2, in_f32=True)
        z1 = [wk.tile([128, QC], bf16, name=f"z1{c}", tag=f"z1{c}")
              for c in range(EC)]
        for fc in range(EC):
            p1 = ps.tile([128, QC], f32, name="ffn", tag="ps")
            for ec in range(EC):
                nc.tensor.matmul(
                    p1[:, :],
                    w_sb["W1"][:, E * ec + 128 * fc:E * ec + 128 * (fc + 1)],
                    ln2[ec][:, :],
                    start=(ec == 0), stop=(ec == EC - 1))
            nc.vector.tensor_scalar(z1[fc][:, :], p1[:, :],
                                    vecs[:, 2 * V_B1 + fc:2 * V_B1 + fc + 1],
                                    0.0, ALU.add, ALU.max)
        for fc in range(EC):
            p2 = ps.tile([128, QC], f32, name="ffn2", tag="ps")
            for ec in range(EC):
                nc.tensor.matmul(
                    p2[:, :],
                    w_sb["W2"][:, E * ec + 128 * fc:E * ec + 128 * (fc + 1)],
                    z1[ec][:, :],
                    start=(ec == 0), stop=(ec == EC - 1))
            of = iop.tile([128, QC], f32, name="of", tag="of")
            nc.vector.affine_then_add(
                of[:, :], p2[:, :], h_sb[qc][fc][:, :],
                1.0, vecs[:, 2 * V_B2 + fc:2 * V_B2 + fc + 1])
            nc.sync.dma_start(
                outT_d[128 * fc:128 * (fc + 1), QC * qc:QC * (qc + 1)],
                of[:, :])

    for p in reversed(persist_pools):
        p.__exit__(None, None, None)


def build_nc():
    nc = bacc.Bacc(
        "TRN2",
        target_bir_lowering=False,
        debug=False,
        enable_asserts=False,
        num_devices=8,
    )
    xT_d = nc.dram_tensor("xT", [E, N], f32, kind="ExternalInput").ap()
    xTb_d = nc.dram_tensor("xTb", [E, N], bf16, kind="ExternalInput").ap()
    inflT_d = nc.dram_tensor("inflT", [N, NQ], bf16, kind="ExternalInput").ap()
    w_d = {
        name: nc.dram_tensor(name, [E, E], bf16, kind="ExternalInput").ap()
        for name in ("Wq", "Wk", "Wv", "Wo", "W1", "W2")
    }
    vecs_d = nc.dram_tensor("vecs", [128, 14], f32, kind="ExternalInput").ap()
    scal_d = nc.dram_tensor("scal", [128, 4], f32, kind="ExternalInput").ap()
    ident_d = nc.dram_tensor("ident", [128, 128], f32, kind="ExternalInput").ap()
    outT_d = nc.dram_tensor("outT", [E, NQ], f32, kind="ExternalOutput").ap()

    with tile.TileContext(nc) as tc:
        build_body(nc, tc, xT_d, xTb_d, inflT_d, w_d, vecs_d, scal_d, ident_d,
                   outT_d)
    nc.compile()
    return nc


def host_shard(inputs):
    """Build the 8 per-core input maps (see module docstring for the roll)."""
    x = np.asarray(inputs["x"], np.float32)
    infl = np.asarray(inputs["influence_matrix"], np.float32)
    vec_list = ["g1", "beta1", "g2", "beta2", "bo", "b1", "b2"]
    vecs_np = np.empty((128, 14), np.float32)
    for vi, nm in enumerate(vec_list):
        v = np.asarray(inputs[nm], np.float32).reshape(E)
        vecs_np[:, 2 * vi] = v[:128]
        vecs_np[:, 2 * vi + 1] = v[128:]
    scal_np = np.tile(
        np.array([inputs["iw1"], inputs["ib1"], inputs["iw2"], inputs["ib2"]],
                 np.float32).reshape(1, 4), (128, 1))
    ws = {n: np.ascontiguousarray(np.asarray(inputs[n], np.float32))
          for n in ("Wq", "Wk", "Wv", "Wo", "W1", "W2")}
    ws["Wq"] = ws["Wq"] / math.sqrt(D)
    ws = {n: w.astype(ml_dtypes.bfloat16) for n, w in ws.items()}

    in_maps = []
    for core in range(8):
        b, qh = core // 2, core % 2
        qoff = qh * NQ
        xb = np.roll(x[b], -qoff, axis=0)          # [N, E], own rows first
        xT = np.ascontiguousarray(xb.T)            # [E, N]
        inf_slice = np.roll(infl[b][qoff:qoff + NQ, :], -qoff, axis=1)
        inflT = np.ascontiguousarray(inf_slice.T)  # [N(k), NQ]
        m = {"xT": xT, "xTb": xT.astype(ml_dtypes.bfloat16),
             "inflT": inflT.astype(ml_dtypes.bfloat16),
             "vecs": vecs_np, "scal": scal_np,
             "ident": np.eye(128, dtype=np.float32)}
        m.update(ws)
        in_maps.append(m)
    return in_maps


_NC_CACHE = []


def kernel(**inputs):
    if not _NC_CACHE:
        _NC_CACHE.append(build_nc())
    nc = _NC_CACHE[0]
    in_maps = host_shard(inputs)
    res = run_bass_kernel_spmd(nc, in_maps, core_ids=list(range(8)))
    out = np.empty((B, N, E), np.float32)
    for core in range(8):
        b, qh = core // 2, core % 2
        out[b, qh * NQ:(qh + 1) * NQ, :] = np.asarray(
            res.results[core]["outT"], np.float32).T
    return out


# revision 12
# speedup vs baseline: 1.0106x; 1.0106x over previous
"""Graphormer layer on 8 TRN2 NeuronCores.

Sharding: core c handles batch b = c//2 and query-row half qh = c%2 (1024 q
rows). All compute is in transposed (feature-on-partition) layout; the host
pre-transposes x and the influence slices and transposes per-core outputs
back during the gather. Host also rolls the node axis per core so each core's
own q rows sit at columns [0, 1024) — the device program is identical across
cores (attention over all keys is permutation-invariant; the influence k axis
is rolled identically).

Stage-E engine balance: per k-chunk a path is chosen from
  'A': influence bias preloaded into PSUM by PE identity-matmul, exact exp
       on the ACT engine.
  'S': same PSUM preload, exp approximated on the Vector engine with a
       Schraudolph bf16 bit-trick (tensor_scalar -> int16, bitcast bf16).
  'P': plain scores; exp on ACT, then multiply by EG = exp(LG) on DVE
       (no PE preload).
The per-(k,q) gate G2 multiply (f = e * G2) always runs on DVE (bf16 2x),
with a configurable subset routed to GpSimd.
"""

import math

import numpy as np
import ml_dtypes

import concourse.bass as bass
import concourse.bacc as bacc
import concourse.mybir as mybir
import concourse.tile as tile
from concourse.bass_utils import run_bass_kernel_spmd

B, N, E, H, D = 4, 2048, 256, 8, 32
NQ = N // 2          # q rows per core
QC = 512             # q window
NKC = N // 128       # 16 k-chunks
EC = E // 128        # 2 feature chunks

f32 = mybir.dt.float32
bf16 = mybir.dt.bfloat16
i16 = mybir.dt.int16
FT = mybir.ActivationFunctionType
ALU = mybir.AluOpType

# Schraudolph constants for bf16 bitcast exp: e ~= bitcast(i16(x*SA + SB))
SA = 128.0 / math.log(2.0)
C_ADJ = 6.0
SB = 127.0 * 128.0 - C_ADJ

# per-kc exp path: 'A' (ACT exp, LG in psum), 'S' (DVE schraudolph, LG in
# psum), 'P' (ACT exp, EG multiply on DVE)
PATHS = "ASPAAPSAAPSAAPAA"
assert len(PATHS) == NKC
# route every FG-th f-multiply to gpsimd (0 = never)
F_GPS_EVERY = 7

# vecs_sb column index: vec v, chunk c -> 2*v + c
V_G1, V_BETA1, V_G2, V_BETA2, V_BO, V_B1, V_B2 = range(7)
# scal columns: iw1, ib1, iw2, ib2
S_IW1, S_IB1, S_IW2, S_IB2 = range(4)


def build_body(nc, tc, xT_d, xTb_d, inflT_d, w_d, vecs_d, scal_d, ident_d,
               outT_d):
    persist_pools = []

    def ppool(name):
        p = tc.tile_pool(name=name, bufs=1)
        persist_pools.append(p)
        return p.__enter__()

    persist = ppool("persist")

    # ---- persistent SBUF ----
    qt = [persist.tile([128, NQ], bf16, name=f"qt{c}", tag=f"qt{c}") for c in range(EC)]
    kt = [persist.tile([128, N], bf16, name=f"kt{c}", tag=f"kt{c}") for c in range(EC)]
    xt = [persist.tile([128, N], f32, name=f"xt{c}", tag=f"xt{c}") for c in range(EC)]
    xb = [persist.tile([128, N], bf16, name=f"xb{c}", tag=f"xb{c}") for c in range(EC)]
    ln1 = [persist.tile([128, N], bf16, name=f"ln1{c}", tag=f"ln1{c}") for c in range(EC)]
    v_sb = [persist.tile([128, E], bf16, name=f"v{k}", tag=f"v{k}") for k in range(NKC)]
    ga_sb = [persist.tile([128, NQ], bf16, name=f"ga_{k}", tag=f"ga_{k}") for k in range(NKC)]
    gb_sb = [persist.tile([128, NQ], bf16, name=f"gb_{k}", tag=f"gb_{k}") for k in range(NKC)]
    id_bf = persist.tile([128, 128], bf16, name="id_bf", tag="id_bf")
    w_sb = {n: persist.tile([128, 2 * E], bf16, name=f"w_{n}", tag=f"w_{n}") for n in w_d}
    vecs = persist.tile([128, 14], f32, name="vecs", tag="vecs")
    scal = persist.tile([128, 4], f32, name="scal", tag="scal")
    ones = persist.tile([128, 128], f32, name="ones", tag="ones")
    ones_bf = persist.tile([128, 32], bf16, name="ones_bf", tag="ones_bf")
    h_sb = [[persist.tile([128, QC], f32, name=f"h{q}{c}", tag=f"h{q}{c}") for c in range(EC)]
            for q in range(2)]
    eps_t = persist.tile([128, 1], f32, name="eps_t", tag="eps_t")
    warm = persist.tile([128, 8], f32, name="warm", tag="warm")

    # ---- ACT table warmup: force exp table load before any data arrives ----
    nc.vector.memset(warm[:, 0:4], 0.0)
    nc.scalar.activation(warm[:, 4:8], warm[:, 0:4], FT.Exp)

    # ---- small loads ----
    for n in w_d:
        for c in range(EC):
            nc.sync.dma_start(w_sb[n][:, E * c:E * (c + 1)],
                              w_d[n][128 * c:128 * (c + 1), :])
    nc.sync.dma_start(vecs[:, :], vecs_d[:, :])
    nc.sync.dma_start(scal[:, :], scal_d[:, :])
    idt = persist.tile([128, 128], f32, name="id_f32", tag="id_f32")
    nc.sync.dma_start(idt[:, :], ident_d[:, :])
    nc.vector.tensor_copy(id_bf[:, :], idt[:, :])
    nc.vector.memset(eps_t[:, :], 1e-5)
    nc.vector.memset(ones[:, :], 1.0)
    nc.vector.memset(ones_bf[:, :], 1.0)

    # ---- x loads (per 512-window so LN can start early) ----
    for w in range(N // 512):
        for c in range(EC):
            nc.sync.dma_start(xt[c][:, 512 * w:512 * (w + 1)],
                              xT_d[128 * c:128 * (c + 1), 512 * w:512 * (w + 1)])
            nc.sync.dma_start(xb[c][:, 512 * w:512 * (w + 1)],
                              xTb_d[128 * c:128 * (c + 1), 512 * w:512 * (w + 1)])

    # ---- influence loads (bf16, per k-chunk) ----
    u_pool = tc.tile_pool(name="u_pool", bufs=3)
    persist_pools.append(u_pool)
    up = u_pool.__enter__()
    u_tiles = {}
    for k in range(NKC):
        u = up.tile([128, NQ], bf16, name=f"u{k}", tag="u")
        nc.sync.dma_start(u[:, :], inflT_d[128 * k:128 * (k + 1), :])
        u_tiles[k] = u

    # ---- shared PSUM pools (4 banks score/general + 4 banks accumulators) --
    ps_pool = tc.tile_pool(name="ps", bufs=2, space="PSUM")
    persist_pools.append(ps_pool)
    ps = ps_pool.__enter__()
    acc_pool = tc.tile_pool(name="acc", bufs=1, space="PSUM")
    persist_pools.append(acc_pool)
    accp = acc_pool.__enter__()
    sbuf_pool = tc.tile_pool(name="work", bufs=1)
    persist_pools.append(sbuf_pool)
    wk = sbuf_pool.__enter__()
    ef_pool = tc.tile_pool(name="ef", bufs=3)
    persist_pools.append(ef_pool)
    efp = ef_pool.__enter__()
    io_pool = tc.tile_pool(name="io", bufs=2)
    persist_pools.append(io_pool)
    iop = io_pool.__enter__()

    ones_lnb = persist.tile([128, 128], bf16, name="ones_lnb", tag="ones_lnb")
    nc.vector.memset(ones_lnb[:, :], 1.0)

    def layer_norm_T(x_chunks, win, wn, g_col, b_col, out_chunks, in_f32):
        """T-layout LN over partition dim; rstd via exp(-0.5*ln(var+eps))."""
        sdt = f32 if in_f32 else bf16
        ones_mm = ones if in_f32 else ones_lnb
        p_s = ps.tile([128, wn], f32, name="lnps", tag="ps")
        for c in range(EC):
            nc.tensor.matmul(p_s[:, :wn], ones_mm[:, :],
                             x_chunks[c][:, win:win + wn],
                             start=(c == 0), stop=(c == EC - 1))
        mu = wk.tile([128, wn], sdt, name="lnmu", tag="lnmu")
        nc.vector.tensor_scalar_mul(mu[:, :], p_s[:, :wn], 1.0 / E)
        mu2 = wk.tile([128, wn], f32, name="lnmu2", tag="lnmu2")
        nc.vector.tensor_mul(mu2[:, :], mu[:, :], mu[:, :])
        sq = wk.tile([128, 2 * wn], sdt, name="lnsq", tag="lnsq")
        p_sq = ps.tile([128, wn], f32, name="lnpsq", tag="ps")
        for c in range(EC):
            xs = x_chunks[c][:, win:win + wn]
            nc.vector.tensor_mul(sq[:, c * wn:(c + 1) * wn], xs, xs)
            nc.tensor.matmul(p_sq[:, :wn], ones_mm[:, :],
                             sq[:, c * wn:(c + 1) * wn],
                             start=(c == 0), stop=(c == EC - 1))
        msq = wk.tile([128, wn], f32, name="lnmsq", tag="lnmsq")
        nc.vector.tensor_scalar_mul(msq[:, :], p_sq[:, :wn], 1.0 / E)
        var = wk.tile([128, wn], f32, name="lnvar", tag="lnvar")
        nc.vector.tensor_sub(var[:, :], msq[:, :], mu2[:, :])
        lnv = wk.tile([128, wn], f32, name="lnlnv", tag="lnlnv")
        nc.scalar.activation(lnv[:, :], var[:, :], FT.Ln, bias=eps_t[:, :])
        rstd = wk.tile([128, wn], sdt, name="lnrstd", tag="lnrstd")
        nc.scalar.activation(rstd[:, :], lnv[:, :], FT.Exp, scale=-0.5)
        for c in range(EC):
            xs = x_chunks[c][:, win:win + wn]
            xm = wk.tile([128, wn], sdt, name="lnxm", tag="lnxm")
            nc.vector.tensor_sub(xm[:, :], xs, mu[:, :])
            xm2 = wk.tile([128, wn], sdt, name="lnxm2", tag="lnxm2")
            nc.vector.tensor_mul(xm2[:, :], xm[:, :], rstd[:, :])
            nc.vector.tensor_scalar(
                out_chunks[c][:, win:win + wn], xm2[:, :],
                vecs[:, 2 * g_col + c:2 * g_col + c + 1],
                vecs[:, 2 * b_col + c:2 * b_col + c + 1],
                ALU.mult, ALU.add)

    # ---- prologue: LN1 w0,w1 -> Q proj; then per kw: K, V, preps ----
    for w in range(2):
        layer_norm_T(xb, 512 * w, 512, V_G1, V_BETA1, ln1, in_f32=False)
    for fc in range(EC):
        for qw in range(NQ // 512):
            pq = ps.tile([128, 512], f32, name="proj", tag="ps")
            for ec in range(EC):
                nc.tensor.matmul(
                    pq[:, :],
                    w_sb["Wq"][:, E * ec + 128 * fc:E * ec + 128 * (fc + 1)],
                    ln1[ec][:, 512 * qw:512 * (qw + 1)],
                    start=(ec == 0), stop=(ec == EC - 1))
            nc.vector.tensor_copy(qt[fc][:, 512 * qw:512 * (qw + 1)], pq[:, :])

    def prep_kc(k):
        u = u_tiles[k]
        path = PATHS[k]
        if path in "AS":
            nc.vector.tensor_scalar(ga_sb[k][:, :], u[:, :], scal[:, 0:1],
                                    scal[:, 1:2], ALU.mult, ALU.add)
        else:  # 'P': EG = exp(iw1*u + ib1)
            nc.scalar.activation(ga_sb[k][:, :], u[:, :], FT.Exp,
                                 scale=scal[:, 0:1], bias=scal[:, 1:2])
        nc.vector.tensor_scalar(gb_sb[k][:, :], u[:, :], scal[:, 2:3],
                                scal[:, 3:4], ALU.mult, ALU.add)

    def proj_kv(kc):
        """Emit K-column-window projection (on kw boundaries), V and influence
        prep for one k-chunk. Interleaved into the qc0 attention loop so no
        engine drains while another catches up (keeps PE's HAM warm)."""
        kw = kc // 4
        if kc % 4 == 0:
            if kw >= 2:
                layer_norm_T(xb, 512 * kw, 512, V_G1, V_BETA1, ln1,
                             in_f32=False)
            for fc in range(EC):
                pk = ps.tile([128, 512], f32, name="proj", tag="ps")
                for ec in range(EC):
                    nc.tensor.matmul(
                        pk[:, :],
                        w_sb["Wk"][:, E * ec + 128 * fc:E * ec + 128 * (fc + 1)],
                        ln1[ec][:, 512 * kw:512 * (kw + 1)],
                        start=(ec == 0), stop=(ec == EC - 1))
                nc.vector.tensor_copy(kt[fc][:, 512 * kw:512 * (kw + 1)],
                                      pk[:, :])
        pv = ps.tile([128, E], f32, name="projv", tag="ps")
        for ec in range(EC):
            nc.tensor.matmul(
                pv[:, :],
                ln1[ec][:, 128 * kc:128 * (kc + 1)],
                w_sb["Wv"][:, E * ec:E * (ec + 1)],
                start=(ec == 0), stop=(ec == EC - 1))
        nc.vector.tensor_copy(v_sb[kc][:, :], pv[:, :])
        prep_kc(kc)

    # ---- attention + per-qc epilogue/FFN ----
    tile_idx = 0
    for qc in range(2):
        q0 = QC * qc
        wv_ps = [accp.tile([128, QC], f32, name=f"wv{qc}{s}", tag=f"wv{s}")
                 for s in range(2)]
        z_ps = [accp.tile([128, QC], f32, name=f"z{qc}{s}", tag=f"z{s}")
                for s in range(2)]
        for kc in range(NKC):
            if qc == 0:
                if kc == 0:
                    proj_kv(0)
                    proj_kv(1)
                if kc + 2 < NKC:
                    proj_kv(kc + 2)
            path = PATHS[kc]
            gab = ga_sb[kc][:, q0:q0 + QC].rearrange(
                "p (o q) -> p o q", o=1).broadcast_to([128, 2, QC])
            gbb = gb_sb[kc][:, q0:q0 + QC].rearrange(
                "p (o q) -> p o q", o=1).broadcast_to([128, 2, QC])
            for half in range(2):
                for hg in (2 * half, 2 * half + 1):
                    st = ps.tile([128, 2 * QC], f32, name="score", tag="ps")
                    if path in "AS":
                        for j in range(2):
                            nc.tensor.matmul(
                                st[:, QC * j:QC * (j + 1)],
                                id_bf[:, :],
                                ga_sb[kc][:, q0:q0 + QC],
                                start=True, stop=False)
                    for j in range(2):
                        h = 2 * hg + j
                        c, hh = h // 4, 32 * (h % 4)
                        nc.tensor.matmul(
                            st[:, QC * j:QC * (j + 1)],
                            kt[c][hh:hh + 32, 128 * kc:128 * (kc + 1)],
                            qt[c][hh:hh + 32, q0:q0 + QC],
                            start=(path == "P"), stop=True,
                            skip_group_check=True, tile_position=(hh, 0))
                    # exp / gating per path
                    if path == "A":
                        e = efp.tile([128, 2 * QC], bf16, name="e", tag="e")
                        nc.scalar.activation(e[:, :], st[:, :], FT.Exp)
                        ztile, zcast = e, False
                    elif path == "S":
                        ei = efp.tile([128, 2 * QC], i16, name="es", tag="e")
                        nc.vector.tensor_scalar(ei[:, :], st[:, :], SA, SB,
                                                ALU.mult, ALU.add)
                        ztile, zcast = ei, True
                    else:  # 'P'
                        e0 = efp.tile([128, 2 * QC], bf16, name="e", tag="e")
                        nc.scalar.activation(e0[:, :], st[:, :], FT.Exp)
                        zt = efp.tile([128, 2 * QC], bf16, name="zt", tag="zt")
                        nc.vector.tensor_tensor(
                            zt[:, :].rearrange("p (o q) -> p o q", o=2),
                            e0[:, :].rearrange("p (o q) -> p o q", o=2),
                            gab, ALU.mult)
                        ztile, zcast = zt, False

                    def zsl(j):
                        ap = ztile[:, QC * j:QC * (j + 1)]
                        return ap.bitcast(bf16) if zcast else ap

                    f = efp.tile([128, 2 * QC], bf16, name="f", tag="f")
                    fsrc = ztile[:, :].bitcast(bf16) if zcast else ztile[:, :]
                    feng = nc.vector
                    tile_idx += 1
                    if F_GPS_EVERY and tile_idx % F_GPS_EVERY == 0:
                        feng = nc.gpsimd
                    feng.tensor_tensor(
                        f[:, :].rearrange("p (o q) -> p o q", o=2),
                        fsrc.rearrange("p (o q) -> p o q", o=2),
                        gbb, ALU.mult)
                    for j in range(2):
                        h = 2 * hg + j
                        s_, hh = h // 4, 32 * (h % 4)
                        nc.tensor.matmul(
                            z_ps[s_][hh:hh + 32, :],
                            ones_bf[:, :],
                            zsl(j),
                            start=(kc == 0), stop=(kc == NKC - 1),
                            skip_group_check=True, tile_position=(0, hh))
                        nc.tensor.matmul(
                            wv_ps[s_][hh:hh + 32, :],
                            v_sb[kc][:, 32 * h:32 * h + 32],
                            f[:, QC * j:QC * (j + 1)],
                            start=(kc == 0), stop=(kc == NKC - 1),
                            skip_group_check=True, tile_position=(0, hh))
        # ---- epilogue: normalize + Wo + residual -> h ----
        on = []
        for s in range(2):
            zr = wk.tile([128, QC], f32, name=f"zr{s}", tag=f"zr{s}")
            nc.vector.reciprocal_approx_fast(zr[:, :], z_ps[s][:, :])
            o = wk.tile([128, QC], bf16, name=f"on{s}", tag=f"on{s}")
            nc.vector.tensor_mul(o[:, :], wv_ps[s][:, :], zr[:, :])
            on.append(o)
        for fc in range(EC):
            po = ps.tile([128, QC], f32, name="po", tag="ps")
            for ec in range(EC):
                nc.tensor.matmul(
                    po[:, :],
                    w_sb["Wo"][:, E * ec + 128 * fc:E * ec + 128 * (fc + 1)],
                    on[ec][:, :],
                    start=(ec == 0), stop=(ec == EC - 1))
            nc.vector.affine_then_add(
                h_sb[qc][fc][:, :], po[:, :], xt[fc][:, q0:q0 + QC],
                1.0, vecs[:, 2 * V_BO + fc:2 * V_BO + fc + 1])
        # ---- stage F: LN2 + FFN + residual + store ----
        ln2 = [wk.tile([128, QC], bf16, name=f"ln2{c}", tag=f"ln2{c}")
               for c in range(EC)]
        layer_norm_T(h_sb[qc], 0, QC, V_G2, V_BETA2, ln2, in_f32=True)
        z1 = [wk.tile([128, QC], bf16, name=f"z1{c}", tag=f"z1{c}")
              for c in range(EC)]
        for fc in range(EC):
            p1 = ps.tile([128, QC], f32, name="ffn", tag="ps")
            for ec in range(EC):
                nc.tensor.matmul(
                    p1[:, :],
                    w_sb["W1"][:, E * ec + 128 * fc:E * ec + 128 * (fc + 1)],
                    ln2[ec][:, :],
                    start=(ec == 0), stop=(ec == EC - 1))
            nc.vector.tensor_scalar(z1[fc][:, :], p1[:, :],
                                    vecs[:, 2 * V_B1 + fc:2 * V_B1 + fc + 1],
                                    0.0, ALU.add, ALU.max)
        for fc in range(EC):
            p2 = ps.tile([128, QC], f32, name="ffn2", tag="ps")
            for ec in range(EC):
                nc.tensor.matmul(
                    p2[:, :],
                    w_sb["W2"][:, E * ec + 128 * fc:E * ec + 128 * (fc + 1)],
                    z1[ec][:, :],
                    start=(ec == 0), stop=(ec == EC - 1))
            of = iop.tile([128, QC], f32, name="of", tag="of")
            nc.vector.affine_then_add(
                of[:, :], p2[:, :], h_sb[qc][fc][:, :],
                1.0, vecs[:, 2 * V_B2 + fc:2 * V_B2 + fc + 1])
            nc.sync.dma_start(
                outT_d[128 * fc:128 * (fc + 1), QC * qc:QC * (qc + 1)],
                of[:, :])

    for p in reversed(persist_pools):
        p.__exit__(None, None, None)


def build_nc():
    nc = bacc.Bacc(
        "TRN2",
        target_bir_lowering=False,
        debug=False,
        enable_asserts=False,
        num_devices=8,
    )
    xT_d = nc.dram_tensor("xT", [E, N], f32, kind="ExternalInput").ap()
    xTb_d = nc.dram_tensor("xTb", [E, N], bf16, kind="ExternalInput").ap()
    inflT_d = nc.dram_tensor("inflT", [N, NQ], bf16, kind="ExternalInput").ap()
    w_d = {
        name: nc.dram_tensor(name, [E, E], bf16, kind="ExternalInput").ap()
        for name in ("Wq", "Wk", "Wv", "Wo", "W1", "W2")
    }
    vecs_d = nc.dram_tensor("vecs", [128, 14], f32, kind="ExternalInput").ap()
    scal_d = nc.dram_tensor("scal", [128, 4], f32, kind="ExternalInput").ap()
    ident_d = nc.dram_tensor("ident", [128, 128], f32, kind="ExternalInput").ap()
    outT_d = nc.dram_tensor("outT", [E, NQ], f32, kind="ExternalOutput").ap()

    with tile.TileContext(nc) as tc:
        build_body(nc, tc, xT_d, xTb_d, inflT_d, w_d, vecs_d, scal_d, ident_d,
                   outT_d)
    nc.compile()
    return nc


def host_shard(inputs):
    """Build the 8 per-core input maps (see module docstring for the roll)."""
    x = np.asarray(inputs["x"], np.float32)
    infl = np.asarray(inputs["influence_matrix"], np.float32)
    vec_list = ["g1", "beta1", "g2", "beta2", "bo", "b1", "b2"]
    vecs_np = np.empty((128, 14), np.float32)
    for vi, nm in enumerate(vec_list):
        v = np.asarray(inputs[nm], np.float32).reshape(E)
        vecs_np[:, 2 * vi] = v[:128]
        vecs_np[:, 2 * vi + 1] = v[128:]
    scal_np = np.tile(
        np.array([inputs["iw1"], inputs["ib1"], inputs["iw2"], inputs["ib2"]],
                 np.float32).reshape(1, 4), (128, 1))
    ws = {n: np.ascontiguousarray(np.asarray(inputs[n], np.float32))
          for n in ("Wq", "Wk", "Wv", "Wo", "W1", "W2")}
    ws["Wq"] = ws["Wq"] / math.sqrt(D)
    ws = {n: w.astype(ml_dtypes.bfloat16) for n, w in ws.items()}

    in_maps = []
    for core in range(8):
        b, qh = core // 2, core % 2
        qoff = qh * NQ
        xb = np.roll(x[b], -qoff, axis=0)          # [N, E], own rows first
        xT = np.ascontiguousarray(xb.T)            # [E, N]
        inf_slice = np.roll(infl[b][qoff:qoff + NQ, :], -qoff, axis=1)
        inflT = np.ascontiguousarray(inf_slice.T)  # [N(k), NQ]
        m = {"xT": xT, "xTb": xT.astype(ml_dtypes.bfloat16),
             "inflT": inflT.astype(ml_dtypes.bfloat16),
             "vecs": vecs_np, "scal": scal_np,
             "ident": np.eye(128, dtype=np.float32)}
        m.update(ws)
        in_maps.append(m)
    return in_maps


_NC_CACHE = []


def kernel(**inputs):
    if not _NC_CACHE:
        _NC_CACHE.append(build_nc())
    nc = _NC_CACHE[0]
    in_maps = host_shard(inputs)
    res = run_bass_kernel_spmd(nc, in_maps, core_ids=list(range(8)))
    out = np.empty((B, N, E), np.float32)
    for core in range(8):
        b, qh = core // 2, core % 2
        out[b, qh * NQ:(qh + 1) * NQ, :] = np.asarray(
            res.results[core]["outT"], np.float32).T
    return out
